# revision 1
# baseline (speedup 1.0000x reference)
"""Trainium2 Bass kernel for nn_BasicRNN_42271068127787.

3-layer LSTM (input=20, hidden=6, seq=34) + FC(204->20) + log_softmax over
batch 32768, data-parallel over 8 NeuronCores (4096 rows/core).

Layout (per core):
  - batch 4096 rows -> NB=10 chunks x BF=416 cols (4160, 64 zero-pad cols)
  - feature-major on chip: activations [feature-rows, batch-cols]
  - gates per (t, layer): one PSUM tile G2 [128, 2, 416] (2 banks):
      bank0 = [i-gates rows 0..59 | g-gates rows 64..123]
      bank1 = [f-gates | o-gates]
    from block-diagonal replicated-weight float32r matmuls (1 cyc/row).
  - walrus rules honored: DVE same-base for two SBUF inputs (outputs and
    ACT outputs may shift partitions; PSUM input exempts the rule).
  - h-state: A [64, 416] = h0 ; B [128, 416] = [h1 | h2] at rows 0/64.
    Row 60 of A/B and row 124 of B hold constant 1.0: biases are folded
    into the recurrent matmul lhsT (bias row 60), fc bias as fc_b/SEQ.
  - FC accumulated inline over t into 2 pinned PSUM tiles [100, 416]
  - log_softmax on device (no max subtraction: logits are O(1))
"""

import sys

import numpy as np

if "/opt/trn_rl_repo" not in sys.path:
    sys.path.insert(0, "/opt/trn_rl_repo")

B_TOTAL = 32768
INPUT = 20
HID = 6
SEQ = 34
CLS = 20
NCORES = 8
BC = B_TOTAL // NCORES  # 4096
NB = 10                 # batch chunks per core
BF = 410                # batch cols per chunk
BCP = NB * BF           # 4160 padded rows per core

_CACHE = {}


# ---------------------------------------------------------------- host prep

def _build_wblob(w_ih, w_hh, b_ih, b_hh, fc_w, fc_b):
    """Pack all lhsT weight tiles into one [128, WC] fp32 blob.

    Gate order in torch weights: rows 0..5=i, 6..11=f, 12..17=g, 18..23=o.
    M-layout of IG tiles: col 6c+h = i-gate, col 64+6c+h = g-gate.
    FO tiles: f / o.  Bias row: lhsT row 60 (paired with const-1.0 row 60
    of the h rhs tiles).
    """
    cols = {}
    blocks = []
    cursor = 0

    def alloc(name, n):
        nonlocal cursor
        cols[name] = cursor
        arr = np.zeros((128, n), dtype=np.float32)
        blocks.append(arr)
        cursor += n
        return arr

    def fill_gate_cols(dst, row_of, src_w, ga, gb, nin):
        for c in range(NB):
            for h in range(HID):
                for k in range(nin):
                    r = row_of(c, k)
                    dst[r, 6 * c + h] = src_w[ga * HID + h, k]
                    dst[r, 64 + 6 * c + h] = src_w[gb * HID + h, k]

    def fill_bias_row(dst, row, bsum, ga, gb):
        for c in range(NB):
            for h in range(HID):
                dst[row, 6 * c + h] = bsum[ga * HID + h]
                dst[row, 64 + 6 * c + h] = bsum[gb * HID + h]

    bsum = [b_ih[l] + b_hh[l] for l in range(3)]

    # layer 0: x feats split 0..9 / 10..19 (chunk-major rows 10c+f), h0 tile
    for half in range(2):
        for nm, ga, gb in (("x%dIG" % half, 0, 2), ("x%dFO" % half, 1, 3)):
            a = alloc(nm, 128)
            fill_gate_cols(a, lambda c, k: 10 * c + k,
                           w_ih[0][:, half * 10:half * 10 + 10], ga, gb, 10)
    for nm, ga, gb in (("hIG0", 0, 2), ("hFO0", 1, 3)):
        a = alloc(nm, 128)
        fill_gate_cols(a, lambda c, k: 6 * c + k, w_hh[0], ga, gb, HID)
        fill_bias_row(a, 60, bsum[0], ga, gb)
    # layer 1: input part (reads A = h0, bias row) and recurrent (reads B[0:64])
    for nm, src, ga, gb, brow in (("aIG1", w_ih[1], 0, 2, True),
                                  ("aFO1", w_ih[1], 1, 3, True),
                                  ("bIG1", w_hh[1], 0, 2, False),
                                  ("bFO1", w_hh[1], 1, 3, False)):
        a = alloc(nm, 128)
        fill_gate_cols(a, lambda c, k: 6 * c + k, src, ga, gb, HID)
        if brow:
            fill_bias_row(a, 60, bsum[1], ga, gb)
    # layer 2 fused: rows 0..63 = h1 block (w_ih2, bias row 60),
    #                rows 64..127 = h2 block (w_hh2)
    for nm, ga, gb in (("W2IG", 0, 2), ("W2FO", 1, 3)):
        a = alloc(nm, 128)
        for c in range(NB):
            for h in range(HID):
                for k in range(HID):
                    a[6 * c + k, 6 * c + h] = w_ih[2][ga * HID + h, k]
                    a[6 * c + k, 64 + 6 * c + h] = w_ih[2][gb * HID + h, k]
                    a[64 + 6 * c + k, 6 * c + h] = w_hh[2][ga * HID + h, k]
                    a[64 + 6 * c + k, 64 + 6 * c + h] = w_hh[2][gb * HID + h, k]
        fill_bias_row(a, 60, bsum[2], ga, gb)
    # FC: rhs is B[64:128] (base 64) -> lhsT tiles live at rows 64..127.
    # Row 124 pairs with B's const-1.0 row: fc bias / SEQ added every t.
    for t in range(SEQ):
        a = alloc("fcA%d" % t, 100)
        b = alloc("fcB%d" % t, 100)
        for c in range(NB):
            for cl in range(10):
                for h in range(HID):
                    a[64 + 6 * c + h, 10 * c + cl] = fc_w[cl, t * HID + h]
                    b[64 + 6 * c + h, 10 * c + cl] = fc_w[10 + cl, t * HID + h]
                a[124, 10 * c + cl] = fc_b[cl] / SEQ
                b[124, 10 * c + cl] = fc_b[10 + cl] / SEQ
    # block-diag ones for per-chunk logsumexp reduce/broadcast
    a = alloc("onesK", 10)      # lhsT [100, 10]: col c = 1 at rows 10c..10c+9
    b = alloc("onesM", 100)     # lhsT [10, 100]: row c = 1 at cols 10c..10c+9
    for c in range(NB):
        a[10 * c:10 * c + 10, c] = 1.0
        b[c, 10 * c:10 * c + 10] = 1.0
    # all-ones row source for the const-1.0 rows of A/B
    a = alloc("ones416", BF)
    a[:] = 1.0

    blob = np.concatenate(blocks, axis=1)
    return np.ascontiguousarray(blob), cols


def _prep_x(x_core):
    """(4096, 20, 34) -> [34, 2, 100, 416] fp32, chunk c col j <-> row c*416+j."""
    xp = np.zeros((BCP, INPUT, SEQ), dtype=np.float32)
    xp[:BC] = x_core
    xr = xp.reshape(NB, BF, INPUT, SEQ).transpose(3, 2, 0, 1)  # (34, 20, 10, 416)
    xr = xr.reshape(SEQ, 2, 10, NB, BF).transpose(0, 1, 3, 2, 4)
    return np.ascontiguousarray(xr.reshape(SEQ, 2, 100, BF))


def _unpack_out(res):
    """[2, 100, 416] -> (4096, 20)."""
    r = res.reshape(2, NB, 10, BF)          # (half, chunk, cls, col)
    r = r.transpose(1, 3, 0, 2).reshape(BCP, CLS)
    return r[:BC]


# ---------------------------------------------------------------- program

def _make_nc(wc_total, col, loop_n=1):
    import concourse.tile as tile
    from concourse import bacc, mybir

    F = mybir.dt.float32
    FR = mybir.dt.float32r
    AF = mybir.ActivationFunctionType
    Alu = mybir.AluOpType

    nc = bacc.Bacc("TRN2", target_bir_lowering=False, debug=False)
    xd = nc.declare_dram_parameter("xin", [SEQ, 2, 100, BF], FR, isOutput=False)
    wd = nc.declare_dram_parameter("win", [128, wc_total], FR, isOutput=False)
    od = nc.declare_dram_parameter("oout", [2, 100, BF], F, isOutput=True)

    with tile.TileContext(nc) as tc:
        with (
            tc.tile_pool(name="w", bufs=1) as wp,
            tc.tile_pool(name="x", bufs=4) as xp,
            tc.tile_pool(name="s", bufs=3) as sp,
            tc.tile_pool(name="st", bufs=1) as st,
            tc.tile_pool(name="g", bufs=3, space="PSUM") as gp,
            tc.tile_pool(name="fc", bufs=1, space="PSUM") as fp,
        ):
            wsb = wp.tile([128, wc_total], FR)
            nc.sync.dma_start(out=wsb[:], in_=wd[:])

            def wap(name, r0, r1, c0, c1):
                c = col[name]
                return wsb[r0:r1, c + c0:c + c1]

            import contextlib
            loop_cm = (tc.For_i(0, loop_n, 1,
                                hint_engines=(mybir.EngineType.PE,
                                              mybir.EngineType.Activation,
                                              mybir.EngineType.DVE,
                                              mybir.EngineType.SP))
                       if loop_n > 1 else contextlib.nullcontext())
            with loop_cm:
                A = st.tile([64, BF], FR, tag="A")
                Bt = st.tile([128, BF], FR, tag="B")
                # X2[l]: bank0 = tanh(g) scratch, bank1 = c state
                X2 = [st.tile([64, 2, BF], F, tag="X2%d" % l, name="X2%d" % l)
                      for l in range(3)]
                nc.vector.memset(A[:].bitcast(F), 0.0)
                nc.vector.memset(Bt[:].bitcast(F), 0.0)
                for l in range(3):
                    nc.vector.memset(X2[l][:], 0.0)
                # const-1.0 rows (bias rows) via tiny SBUF->SBUF DMAs
                nc.sync.dma_start(out=A[60:61, :], in_=wap("ones416", 60, 61, 0, BF))
                nc.sync.dma_start(out=Bt[60:61, :], in_=wap("ones416", 60, 61, 0, BF))
                nc.sync.dma_start(out=Bt[124:125, :], in_=wap("ones416", 124, 125, 0, BF))
                pa = fp.tile([100, BF], F, tag="pa")
                pb = fp.tile([100, BF], F, tag="pb")

                hdst = {0: A[0:60], 1: Bt[0:60], 2: Bt[64:124]}
                # wavefront: stage s runs layer l at t = s - l (independent
                # chains); all matmuls first (they read last stage's h), then
                # the elementwise chains, then FC on the just-written h2.
                for s_ in range(SEQ + 2):
                    if s_ < SEQ:
                        xa = xp.tile([100, BF], FR, tag="xa")
                        xb = xp.tile([100, BF], FR, tag="xb")
                        nc.sync.dma_start(out=xa[:], in_=xd[s_, 0])
                        nc.sync.dma_start(out=xb[:], in_=xd[s_, 1])
                    rhs_sets = {
                        0: [(xa[:], "x0IG", "x0FO", 100),
                            (xb[:], "x1IG", "x1FO", 100),
                            (A[:], "hIG0", "hFO0", 64)],
                        1: [(A[:], "aIG1", "aFO1", 64),
                            (Bt[0:64], "bIG1", "bFO1", 64)],
                        2: [(Bt[:], "W2IG", "W2FO", 128)],
                    }
                    live = [l for l in range(3) if 0 <= s_ - l < SEQ]
                    g2s = {}
                    for l in live:
                        g2 = gp.tile([128, 2, 512], F, tag="g2",
                                     name="g2_%d_%d" % (s_, l))
                        g2s[l] = g2
                        items = rhs_sets[l]
                        n = len(items)
                        for gi in range(2):
                            for i, (rhs, wig, wfo, K) in enumerate(items):
                                nc.tensor.matmul(g2[:, gi, 0:BF],
                                                 wap(wig if gi == 0 else wfo,
                                                     0, K, 0, 128),
                                                 rhs,
                                                 start=(i == 0),
                                                 stop=(i == n - 1))
                    for l in live:
                        g2 = g2s[l]
                        sif = sp.tile([64, 2, BF], F, tag="sif")
                        so = sp.tile([64, BF], F, tag="so")
                        z = sp.tile([64, 2, BF], F, tag="z")
                        tcl = sp.tile([64, BF], F, tag="tcl")
                        # sigmoid(i | f) in one shot (banks 0,1 of rows 0..63)
                        nc.scalar.activation(out=sif[:], in_=g2[0:64, :, 0:BF],
                                             func=AF.Sigmoid)
                        # tanh(g): rows 64..127 bank0 -> shifted to X2 bank0
                        nc.scalar.activation(out=X2[l][:, 0, :],
                                             in_=g2[64:128, 0, 0:BF],
                                             func=AF.Tanh)
                        # sigmoid(o): rows 64..127 bank1 -> shifted to 0
                        nc.scalar.activation(out=so[:], in_=g2[64:128, 1, 0:BF],
                                             func=AF.Sigmoid)
                        # z = [i*tanh_g | f*c] in one 2-bank op
                        nc.vector.tensor_mul(out=z[:], in0=sif[:], in1=X2[l][:])
                        nc.vector.tensor_add(out=X2[l][:, 1, :],
                                             in0=z[:, 0, :], in1=z[:, 1, :])
                        nc.scalar.activation(out=tcl[:], in_=X2[l][:, 1, :],
                                             func=AF.Tanh)
                        nc.vector.tensor_mul(out=hdst[l], in0=so[0:60],
                                             in1=tcl[0:60])
                    t2_ = s_ - 2
                    if 0 <= t2_ < SEQ:
                        nc.tensor.matmul(pa[:], wap("fcA%d" % t2_, 64, 128, 0, 100),
                                         Bt[64:128],
                                         start=(t2_ == 0), stop=(t2_ == SEQ - 1))
                        nc.tensor.matmul(pb[:], wap("fcB%d" % t2_, 64, 128, 0, 100),
                                         Bt[64:128],
                                         start=(t2_ == 0), stop=(t2_ == SEQ - 1))

                # ---- log_softmax tail (logits are O(1); skip max subtraction)
                ea = sp.tile([100, BF], FR, tag="sif")
                eb = sp.tile([100, BF], FR, tag="tg")
                la = sp.tile([100, BF], F, tag="la")
                lb = sp.tile([100, BF], F, tag="lb")
                nc.scalar.activation(out=la[:], in_=pa[:], func=AF.Identity)
                nc.scalar.activation(out=lb[:], in_=pb[:], func=AF.Identity)
                nc.scalar.activation(out=ea[:], in_=pa[:], func=AF.Exp)
                nc.scalar.activation(out=eb[:], in_=pb[:], func=AF.Exp)
                s = gp.tile([10, BF], F, tag="g2")
                nc.tensor.matmul(s[:], wap("onesK", 0, 100, 0, 10), ea[:],
                                 start=True, stop=False)
                nc.tensor.matmul(s[:], wap("onesK", 0, 100, 0, 10), eb[:],
                                 start=False, stop=True)
                lnz = sp.tile([10, BF], FR, tag="lnz")
                nc.scalar.activation(out=lnz[:], in_=s[:], func=AF.Ln)
                bc = gp.tile([100, BF], F, tag="g2")
                nc.tensor.matmul(bc[:], wap("onesM", 0, 10, 0, 100), lnz[:],
                                 start=True, stop=True)
                oa = sp.tile([100, BF], F, tag="la")
                ob = sp.tile([100, BF], F, tag="lb")
                nc.vector.scalar_tensor_tensor(out=oa[:], in0=bc[:], scalar=-1.0,
                                               in1=la[:], op0=Alu.mult, op1=Alu.add)
                nc.vector.scalar_tensor_tensor(out=ob[:], in0=bc[:], scalar=-1.0,
                                               in1=lb[:], op0=Alu.mult, op1=Alu.add)
                nc.sync.dma_start(out=od[0], in_=oa[:])
                nc.sync.dma_start(out=od[1], in_=ob[:])
    nc.compile()
    return nc


def _get_program(inputs, loop_n=1):
    w_ih = [inputs["w_ih%d" % l] for l in range(3)]
    w_hh = [inputs["w_hh%d" % l] for l in range(3)]
    b_ih = [inputs["b_ih%d" % l] for l in range(3)]
    b_hh = [inputs["b_hh%d" % l] for l in range(3)]
    blob, col = _build_wblob(w_ih, w_hh, b_ih, b_hh,
                             inputs["fc_w"], inputs["fc_b"])
    key = "nc%d" % loop_n
    if key not in _CACHE:
        _CACHE[key] = _make_nc(blob.shape[1], col, loop_n)
    return _CACHE[key], blob


def kernel(**inputs):
    from concourse.bass_utils import run_bass_kernel_spmd

    nc, blob = _get_program(inputs)
    x = np.asarray(inputs["x"], dtype=np.float32)
    in_maps = []
    for c in range(NCORES):
        xc = x[c * BC:(c + 1) * BC, 0]  # (4096, 20, 34)
        in_maps.append({"xin": _prep_x(xc), "win": blob})
    res = run_bass_kernel_spmd(nc, in_maps, list(range(NCORES)),
                               trace=_CACHE.get("trace", False))
    _CACHE["last_res"] = res
    out = np.empty((B_TOTAL, CLS), dtype=np.float32)
    for c in range(NCORES):
        out[c * BC:(c + 1) * BC] = _unpack_out(res.results[c]["oout"])
    return out

